# revision 77
# baseline (speedup 1.0000x reference)
"""FAVOR+ (Performer) non-causal linear attention on 8 Trainium2 NeuronCores.

Sharding: data-parallel over batch B=8 -> one batch element per core.
Per-core pipeline (L=4096, DIM=768, H=12, D=64, M=256):

  prep : k-weight blocks 0-3 DMA'd + fp8-converted FIRST (their ACT/DVE
         copy chain gates the first kT), then x chunk-0 per-subtile with
         subtile-outer transposes into dedicated psum banks (each x
         subtile transposes the moment its DMA lands); k blocks 4-5 are
         DMA'd in prep but transposed inside chunk 0 behind kT(0..3);
         v/q/proj weight sections ride inside chunk 0
  fp8  : the three qkv GEMMs run as fp8e4m3 DoubleRow matmuls (0.5
         cycles/row) with 3-term hi/lo error compensation:
         W'x ~ Whi'xhi + (Whi'xlo + Wlo'xhi), weights prescaled by 32 so
         the residual stays out of the denormal range (the 1/32 rides on
         the psum-readout ACT scale); adds ~2e-4 relative error
  pass1: per 512-row chunk: transpose x -> hi/lo fp8 xT (steady-state
         chunks stage x as bf16 via the idle Pool engine first: bf16
         transposes run 1.0 c/r vs f32r's 1.5); kT for all 6 pairs; then
         v subtiles interleaved with the kpz/qT pipeline so the DVE kp
         relus spread across the whole chunk instead of piling up in a
         pair-loop tail drain; kpz computes both heads of a pair in ONE
         matmul via a block-diagonal pmT (one accumulation group, one
         psum bank); kp and vsb are bf16 (halves their SBUF, matmul
         cost unchanged); kv accumulated per pair, kv_sb += on DVE.
         Last chunk: pair-5 relus deferred between the kv emissions so
         the kv_sb adds don't queue behind them on DVE, kvm transposes
         ride the kv tail, and the kv psum alternates with a borrowed
         kpz bank so back-to-back kvs don't serialize on one bank
  mid  : PE-transpose kv -> m-major [m, d+1]
  pass2: q_p = relu(z+eps) on ACT; num+den fused in one 65-row matmul;
         recip (DVE, drains den from psum) -> partition-broadcast of the
         reciprocal on Pool (sbuf->sbuf only: neither Pool nor DMA may
         touch PSUM on real HW) -> mul on DVE, deferred a few units so
         every op's operands are long produced; y-projection
         half-subtiles of the previous chunk interleaved between units
         (starting at unit 4, after the previous chunk's division
         pipeline fully flushed) as PE filler; one unit of qpz lookahead

  PSUM discipline: concurrent matmul groups never share a bank (hardware
  hazard -- verified empirically; CoreSim does not model it). Every PSUM
  byte must drain through ACT or DVE (GPSIMD/Pool and DMA cannot access
  PSUM -- the bir verifier rejects it).
"""

import math
import os
import sys
from contextlib import ExitStack

import numpy as np

for _p in ("/opt/trn_rl_repo",):
    if _p not in sys.path and os.path.isdir(_p):
        sys.path.insert(0, _p)

import concourse.bass as bass  # noqa: E402
import concourse.mybir as mybir  # noqa: E402
import concourse.tile as tile  # noqa: E402
from concourse import bacc  # noqa: E402

P = 128
DIM = 768
H = 12
D = 64
M = 256
KT = DIM // P  # 6 contraction k-tiles
NPAIR = H // 2  # 6 head pairs; one 128-row feature tile = 2 heads
EPS = 1e-3
RATIO = 1.0 / math.sqrt(float(M))

F32 = mybir.dt.float32
F32R = mybir.dt.float32r
F8 = mybir.dt.float8e4
BF16 = mybir.dt.bfloat16
AL = mybir.AluOpType
AF = mybir.ActivationFunctionType
DR = mybir.MatmulPerfMode.DoubleRow

# fp8 weight scale: W*32 keeps the fp8 residual (W - fp8(W)) out of the
# e4m3 denormal range; the matching 1/32 rides on the psum-readout ACT ops
WS = 32.0
WSI = 1.0 / WS
# development bisect switches, fixed at the shipping configuration
USE_FP8 = True      # fp8 DoubleRow qkv GEMMs with 3-term hi/lo compensation
QTPF_EARLY = True   # qt reload DMAs issued during pass 1
P2OLD = False       # pass-2 unit pipeline with lookahead + y interleave
P1OLD = False       # pass-1 kpz/qT/kv software pipeline
PREPOLD = False     # weight prep interleaved into chunk 0
P1PSOLD = False
P1KT2 = False
K_KEEP = int(os.environ.get("K_KEEP", "1"))   # pend divide deferral
K_DEF = int(os.environ.get("K_DEF", "3"))     # bcast deferral (rqueue len)
K_QPPS = int(os.environ.get("K_QPPS", "3"))   # qppsum bufs
K_NMPS = int(os.environ.get("K_NMPS", "3"))   # numpsum bufs
K_RDP = int(os.environ.get("K_RDP", "12"))     # rdb sbuf bufs
K_YOFF = int(os.environ.get("K_YOFF", "4"))   # y-half unit offset
K_KPACT = int(os.environ.get("K_KPACT", "0"))  # kp relus on ACT for s < this
K_KVLAG = int(os.environ.get("K_KVLAG", "1"))  # kv deferral in pairs
K_KPP = int(os.environ.get("K_KPP", "12"))      # kp sbuf ring bufs
K_KPPS = int(os.environ.get("K_KPPS", "4"))    # kpz psum ring (1-bank tiles)
K_KVPS = int(os.environ.get("K_KVPS", "1"))    # kv psum bufs
K_X0 = int(os.environ.get("K_X0", "1"))       # chunk-0 subtile-outer transposes
K_TRP0 = int(os.environ.get("K_TRP0", "3"))   # prep psum ring bufs
K_X0X = int(os.environ.get("K_X0X", "5"))     # k-tiles with dedicated x0 banks
K_ATP = int(os.environ.get("K_ATP", "2"))     # attn tile ring
K_ALTKV = int(os.environ.get("K_ALTKV", "1"))  # borrow kpz bank for kv tail
K_QTP = int(os.environ.get("K_QTP", "9"))      # pass-2 qt prefetch ring
K_KTP = int(os.environ.get("K_KTP", "6"))      # kt ring
K_QTSB = int(os.environ.get("K_QTSB", "4"))    # qt staging ring
K_YC0 = int(os.environ.get("K_YC0", "0"))      # ysb ci0 readout: 0=ACT 1=DVE
K_YC1 = int(os.environ.get("K_YC1", "0"))      # ysb ci1 readout: 0=DVE 1=ACT
K_XP = int(os.environ.get("K_XP", "3"))        # x/weight staging ring
K_XTP = int(os.environ.get("K_XTP", "2"))      # xT fp8 ring
K_XBF = int(os.environ.get("K_XBF", "2"))      # x bf16 staging ring
SC = WSI if USE_FP8 else 1.0


def _r(ap):
    return ap.bitcast(F32R)


def build(L=4096, has_qkv_b=True, has_proj_b=True):
    LCH = 512
    NCH = L // LCH
    NSUB = LCH // P  # 4

    nc = bacc.Bacc("TRN2", target_bir_lowering=False, debug=False)
    x_d = nc.dram_tensor("x", [L, DIM], F32, kind="ExternalInput").ap()
    qkvw_d = nc.dram_tensor("qkv_w", [3 * DIM, DIM], F32, kind="ExternalInput").ap()
    qkvb_d = nc.dram_tensor("qkv_b", [3 * DIM], F32, kind="ExternalInput").ap()
    projw_d = nc.dram_tensor("proj_w", [DIM, DIM], F32, kind="ExternalInput").ap()
    projb_d = nc.dram_tensor("proj_b", [DIM], F32, kind="ExternalInput").ap()
    pm_d = nc.dram_tensor("proj_mat", [M, D], F32, kind="ExternalInput").ap()
    y_d = nc.dram_tensor("y", [L, DIM], F32, kind="ExternalOutput").ap()

    with tile.TileContext(nc) as tc:
        with ExitStack() as ctx:
            _body(ctx, tc, x_d, qkvw_d, qkvb_d, projw_d, projb_d, pm_d, y_d,
                  L, LCH, NCH, NSUB, has_qkv_b, has_proj_b)
    nc.compile()
    return nc


def _body(ctx, tc, x_d, qkvw_d, qkvb_d, projw_d, projb_d, pm_d, y_d,
          L, LCH, NCH, NSUB, has_qkv_b, has_proj_b):
    nc = tc.nc

    persist = ctx.enter_context(tc.tile_pool(name="persist", bufs=1))

    ident = persist.tile([P, P], F32R, tag="ident", name="ident")[:]
    nc.gpsimd.memset(ident.bitcast(F32), 0.0)
    nc.gpsimd.affine_select(
        out=ident, in_=ident, compare_op=AL.not_equal, fill=1.0,
        base=0, pattern=[[-1, P]], channel_multiplier=1,
    )

    # constant-1 row via ACT (memset can't write f32r): 1.0 = ident*0 + 1
    ones_row = persist.tile([1, P], F32R, tag="ones_row", name="ones_row")[:]
    nc.scalar.activation(ones_row, ident.bitcast(F32)[0:1, :], AF.Copy,
                         bias=1.0, scale=0.0)
    # per-partition eps column: bias operand for the ACT relu(z+eps) ops
    epsc = persist.tile([P, 1], F32, tag="epsc", name="epsc")[:]
    nc.gpsimd.memset(epsc, EPS)
    # bf16 identity: bf16 transposes run at 1.0 cycles/row vs f32r's 1.5
    identb = persist.tile([P, P], BF16, tag="identb", name="identb")[:]
    nc.scalar.copy(identb, ident.bitcast(F32))

    # per-partition q/k biases: qkb[:, t] = qkv_b[t*128 : (t+1)*128], t in 0..11
    # (DMAs issued inside prep, after the startup-critical x/weight blocks)
    qkb = persist.tile([P, 2 * KT], F32, tag="qkb", name="qkb")[:]
    # v bias and proj bias as single rows (used as K=1 matmul rhs);
    # vb32 = WS*vb so the bias survives the 1/WS psum-readout scale
    vb_row = persist.tile([1, DIM], F32R, tag="vb_row", name="vb_row")[:]
    vb32 = persist.tile([1, DIM], F32R, tag="vb32", name="vb32")[:]
    pb_row = persist.tile([1, DIM], F32R, tag="pb_row", name="pb_row")[:]

    # transposed qkv weights, feature-major, fp8 hi/lo split (scaled by WS):
    # wint[k, kk, 0, c] = fp8(WS * qkv_w[c, 128*kk + k]), slot 1 = residual.
    # Slot pairs feed DoubleRow matmuls: (hi,hi) k-tile pairs for the main
    # term, (hi,lo) against x's (lo,hi) for the cross terms.
    if USE_FP8:
        wint = persist.tile([P, KT, 2, 3 * DIM], F8, tag="wint", name="wint")[:]
        qkvwT = None
    else:
        wint = None
        qkvwT = [persist.tile([P, 3 * DIM], F32R, tag=f"qkvwT{kk}",
                              name=f"qkvwT{kk}")[:] for kk in range(KT)]
    projwT = [persist.tile([P, DIM], F32R, tag=f"projwT{kk}",
                           name=f"projwT{kk}")[:] for kk in range(KT)]
    # pmT stacked twice on partitions: rows 0:64 and 64:128 both = RATIO * proj_mat.T
    pmT = persist.tile([P, M], F32R, tag="pmT", name="pmT")[:]
    # block-diagonal variant for the kp features: one matmul computes both
    # heads of a pair into one psum bank (contraction 128, out [128L, 2*M])
    pmTbd = persist.tile([P, 2 * M], F32R, tag="pmTbd", name="pmTbd")[:]
    nc.gpsimd.memset(pmTbd.bitcast(F32), 0.0)
    # kv m-major per pair: kvm[p][m, j, :] with j = 2*h2+mt -> [128 m, 65]
    kvm = [persist.tile([P, 4, D + 1], F32R, tag=f"kvm{p}", name=f"kvm{p}")[:] for p in range(NPAIR)]
    # v chunk buffer (L-major, ones column at d=64 per head written once)
    vsb = persist.tile([P, NSUB, H, D + 1], BF16, tag="vsb", name="vsb")[:]
    nc.scalar.activation(
        vsb[:, :, :, D : D + 1],
        ident.bitcast(F32)[:, 0 : NSUB * H].rearrange(
            "q (s h) -> q s h", s=NSUB
        ).unsqueeze(3),
        AF.Copy, bias=1.0, scale=0.0,
    )

    # qT staged via DRAM between passes; q_p recomputed in pass 2
    qt_dram = ctx.enter_context(tc.tile_pool(name="qtd", bufs=1, space="DRAM"))
    qtd = qt_dram.tile([NCH, NPAIR, P, LCH], F32, tag="qtd", name="qtd")[:]

    # pass-2 qt tiles: pool spans pass 1 (prefetch of early chunks) + pass 2
    qtp = ctx.enter_context(tc.tile_pool(name="p2qt", bufs=K_QTP))
    qt_tiles = {}

    def qt_prefetch(ich):
        for p in range(NPAIR):
            qt = qtp.tile([P, LCH], F32R, tag="qt", name="qt")[:]
            nc.sync.dma_start(qt, _r(qtd[ich, p]))
            qt_tiles[(ich, p)] = qt

    with tc.tile_pool(name="p1x", bufs=K_XP) as xp, \
         tc.tile_pool(name="p1xt", bufs=K_XTP) as xtp, \
         tc.tile_pool(name="p1xbf", bufs=K_XBF) as xbfp:
        wnat_pool = xp  # weight staging reuses the x-prefetch ring (same shape)

        xnats = {}
        tr_pool = []  # psum pool for transposes: trp0 during prep, ktpsum after

        def x_prefetch(ich):
            # one DMA per 128-row subtile: the DMA engine is serial, so finer
            # grain lets the first transposes start ~3x earlier
            xnat = xp.tile([P, NSUB, DIM], F32R, tag="xnat", name="xnat")[:]
            l0 = ich * LCH
            if ich == 0 and not PREPOLD:
                nc.sync.dma_start(xnat[:, 0, 0:384],
                                  _r(x_d[0:P, 0:384]))
                nc.sync.dma_start(xnat[:, 0, 384:DIM],
                                  _r(x_d[0:P, 384:DIM]))
                for s in range(1, NSUB):
                    nc.sync.dma_start(xnat[:, s, :],
                                      _r(x_d[s * P : (s + 1) * P, :]))
                xnats[ich] = xnat
                return
            if PREPOLD:
                nc.sync.dma_start(
                    xnat, _r(x_d[l0 : l0 + LCH, :].rearrange(
                        "(s p) k -> p s k", p=P)))
            else:
                for s in range(NSUB):
                    nc.sync.dma_start(
                        xnat[:, s, :],
                        _r(x_d[l0 + s * P : l0 + (s + 1) * P, :]),
                    )
            xnats[ich] = xnat

        def xt_transposes(ich):
            # xT in fp8 hi/lo split: slot 1 = fp8(x), slot 0 = x - fp8(x)
            if USE_FP8:
                xint = xtp.tile([P, KT, 2, LCH], F8, tag="xt", name="xint")[:]
            else:
                xint = xtp.tile([P, KT, LCH], F32R, tag="xt", name="xint")[:]
            xnat = xnats.pop(ich)
            if USE_FP8 and ich > 0:
                # steady-state chunks: stage x as bf16 on the idle Pool
                # engine so the PE transposes run at 1.0 c/r instead of 1.5
                # (x is re-quantized to fp8 hi/lo right after anyway)
                xbf = xbfp.tile([P, NSUB, DIM], BF16, tag="xbf", name="xbf")[:]
                nc.gpsimd.tensor_copy(xbf, xnat.bitcast(F32))
                for kk in range(KT):
                    psb = tr_pool[0].tile([P, 1024], BF16, tag="ktps",
                                          name="trpsb")[:]
                    for s in range(NSUB):
                        nc.tensor.transpose(
                            psb[:, s * P : (s + 1) * P],
                            xbf[:, s, kk * P : (kk + 1) * P],
                            identb,
                        )
                    nc.scalar.copy(xint[:, kk, 1, :], psb[:, 0:LCH])
                    nc.vector.tensor_sub(xint[:, kk, 0, :], psb[:, 0:LCH],
                                         xint[:, kk, 1, :])
                return xint
            if ich == 0 and K_X0:
                # chunk 0 (startup-critical): subtile-outer order for the
                # first K_X0X k-tiles (one psum bank each), so PE transposes
                # each x subtile the moment its DMA lands; remaining k-tiles
                # follow in the regular ring just in time for their DR pair
                nkx = K_X0X
                pss = [tr_pool[0].tile([P, 512], F32, tag="x0ps", bufs=nkx,
                                       name=f"trps{kk}")[:] for kk in range(nkx)]
                for s in range(NSUB):
                    for kk in range(nkx):
                        nc.tensor.transpose(
                            _r(pss[kk][:, s * P : (s + 1) * P]),
                            _r(xnat[:, s, kk * P : (kk + 1) * P]),
                            _r(ident),
                        )
                for kk in range(nkx):
                    nc.scalar.copy(xint[:, kk, 1, :], pss[kk][:, 0:LCH])
                    nc.vector.tensor_sub(xint[:, kk, 0, :],
                                         pss[kk][:, 0:LCH],
                                         xint[:, kk, 1, :])
                for kk in range(nkx, KT):
                    ps = tr_pool[0].tile([P, 512], F32, tag="ktps",
                                         name="trps")[:]
                    for s in range(NSUB):
                        nc.tensor.transpose(
                            _r(ps[:, s * P : (s + 1) * P]),
                            _r(xnat[:, s, kk * P : (kk + 1) * P]),
                            _r(ident),
                        )
                    nc.scalar.copy(xint[:, kk, 1, :], ps[:, 0:LCH])
                    nc.vector.tensor_sub(xint[:, kk, 0, :], ps[:, 0:LCH],
                                         xint[:, kk, 1, :])
                return xint
            for kk in range(KT):
                ps = tr_pool[0].tile([P, 512], F32, tag="ktps", name="trps")[:]
                for s in range(NSUB):
                    nc.tensor.transpose(
                        _r(ps[:, s * P : (s + 1) * P]),
                        _r(xnat[:, s, kk * P : (kk + 1) * P]),
                        _r(ident),
                    )
                if USE_FP8:
                    nc.scalar.copy(xint[:, kk, 1, :], ps[:, 0:LCH])
                    nc.vector.tensor_sub(xint[:, kk, 0, :], ps[:, 0:LCH],
                                         xint[:, kk, 1, :])
                else:
                    nc.scalar.copy(xint[:, kk, 0:256], ps[:, 0:256])
                    nc.vector.tensor_copy(xint[:, kk, 256:LCH], ps[:, 256:LCH])
            return xint

        # ---- prep: x chunk 0 first, then weight sections just-in-time;
        # v/q/proj sections are emitted inside chunk 0 to overlap compute ----
        with tc.tile_pool(name="trprep", bufs=K_TRP0, space="PSUM") as trp0:

            tr_pool.append(trp0)
            if int(os.environ.get("K_KG1F", "1")):
                pass
            else:
                x_prefetch(0)

            def transpose_into(src, coff, nrows, proj=False, bs_max=4):
                # src [nrows, DIM] DRAM, transposed per 128x128 block.
                # proj: write the fp8 hi/lo split of WS*src into pw8
                # (slot 1 = hi, slot 0 = residual);
                # otherwise writes the fp8 hi/lo split of WS*src into wint.
                nblk = nrows // P
                b0 = 0
                while b0 < nblk:
                    bs = min(bs_max, nblk - b0)
                    wnat = wnat_pool.tile([P, 4, DIM], F32R, tag="xnat", name="wnat")[:]
                    if PREPOLD:
                        nc.sync.dma_start(
                            wnat[:, 0:bs, :],
                            _r(src[b0 * P : (b0 + bs) * P, :].rearrange(
                                "(s p) k -> p s k", p=P)))
                    else:
                        for j in range(bs):
                            nc.sync.dma_start(
                                wnat[:, j, :],
                                _r(src[(b0 + j) * P : (b0 + j + 1) * P, :]),
                            )
                    for kk in range(KT):
                        ps = tr_pool[0].tile([P, 512], F32, tag="ktps", name="trps")[:]
                        for j in range(bs):
                            nc.tensor.transpose(
                                _r(ps[:, j * P : (j + 1) * P]),
                                _r(wnat[:, j, kk * P : (kk + 1) * P]),
                                _r(ident),
                            )
                        cs = slice(coff + b0 * P, coff + (b0 + bs) * P)
                        if proj:
                            if kk % 2 == 0:
                                nc.scalar.copy(projwT[kk][:, cs],
                                               ps[:, 0 : bs * P])
                            else:
                                nc.vector.tensor_copy(projwT[kk][:, cs],
                                                      ps[:, 0 : bs * P])
                        elif USE_FP8:
                            nc.scalar.activation(wint[:, kk, 0, cs],
                                                 ps[:, 0 : bs * P], AF.Copy,
                                                 scale=WS)
                            nc.vector.scalar_tensor_tensor(
                                wint[:, kk, 1, cs], ps[:, 0 : bs * P], WS,
                                wint[:, kk, 0, cs], AL.mult, AL.subtract)
                        else:
                            if kk % 2 == 0:
                                nc.scalar.copy(qkvwT[kk][:, cs], ps[:, 0 : bs * P])
                            else:
                                nc.vector.tensor_copy(qkvwT[kk][:, cs],
                                                      ps[:, 0 : bs * P])
                    b0 += bs

            # chunk-0 x transposes trickle in behind the per-subtile DMAs,
            # then weight sections in use order: k, pm, v, q; proj_w last.
            # k blocks 4-5 are DMA'd here (queue position) but transposed
            # inside chunk 0, after kT(0..3) is emitted -- PE is in-order,
            # so this lets the first kT matmuls run while the prep copy
            # chain for the later blocks is still draining ACT/DVE
            _kg1f = int(os.environ.get("K_KG1F", "1"))
            if _kg1f == 2:
                transpose_into(qkvw_d[DIM : DIM + 256], DIM, 256)
                x_prefetch(0)
                transpose_into(qkvw_d[DIM + 256 : 2 * DIM - 256], DIM + 256,
                               256)
                xt0 = xt_transposes(0)
            elif _kg1f == 1:
                # k blocks 0-3 first in the DMA queue and the ACT/DVE copy
                # chain: kT(0..3) then only waits on x0's transposes
                transpose_into(qkvw_d[DIM : 2 * DIM - 256], DIM, 512,
                               bs_max=int(os.environ.get("K_KBS", "4")))
                x_prefetch(0)
                xt0 = xt_transposes(0)
            else:
                xt0 = xt_transposes(0)
                transpose_into(qkvw_d[DIM : 2 * DIM - 256], DIM, 512)
            kg2 = xp.tile([P, 4, DIM], F32R, tag="xnat", name="kg2")[:]
            for j in range(2):
                nc.sync.dma_start(
                    kg2[:, j, :],
                    _r(qkvw_d[2 * DIM - 256 + j * P : 2 * DIM - 128 + j * P, :]),
                )

            nc.sync.dma_start(
                qkb, qkvb_d.rearrange("(t p) -> p t", p=P)[:, 0 : 2 * KT])
            nc.sync.dma_start(vb_row, _r(qkvb_d[2 * DIM : 3 * DIM].unsqueeze(0)))
            nc.sync.dma_start(pb_row, _r(projb_d.unsqueeze(0)))
            nc.scalar.activation(vb32, vb_row.bitcast(F32), AF.Copy, scale=WS)

            pmn = xp.tile([P, 2, D], F32R, tag="pmn", name="pmn")[:]
            nc.sync.dma_start(pmn, _r(pm_d.rearrange("(s p) d -> p s d", p=P)))
            ps = trp0.tile([P, 512], F32, tag="ktps", name="trps")[:]
            for s in range(2):
                nc.tensor.transpose(
                    _r(ps[0:D, s * P : (s + 1) * P]), _r(pmn[:, s, :]), _r(ident)
                )
            nc.scalar.mul(pmT[0:D, :], ps[0:D, 0:M], RATIO)
            nc.scalar.mul(pmT[D:P, :], ps[0:D, 0:M], RATIO)
            nc.scalar.mul(pmTbd[0:D, 0:M], ps[0:D, 0:M], RATIO)
            nc.scalar.mul(pmTbd[D:P, M : 2 * M], ps[0:D, 0:M], RATIO)

            if PREPOLD:
                transpose_into(qkvw_d[2 * DIM : 3 * DIM], 2 * DIM, DIM)
                transpose_into(qkvw_d[0:DIM], 0, DIM)
                transpose_into(projw_d, 0, DIM, proj=True)

        # ---- pass 1: kv accumulation (+ qT staging as pipeline filler) ----
        with tc.tile_pool(name="p1kt", bufs=K_KTP) as ktp, \
             tc.tile_pool(name="p1qtsb", bufs=K_QTSB) as qtsbp, \
             tc.tile_pool(name="p1kp", bufs=K_KPP) as kpp, \
             tc.tile_pool(name="p1kv", bufs=1) as kvsb_pool, \
             tc.tile_pool(name="ps1kt", bufs=3, space="PSUM") as ktpsum, \
             tc.tile_pool(name="ps1kp", bufs=K_KPPS, space="PSUM") as kppsum, \
             tc.tile_pool(name="ps1kv", bufs=K_KVPS, space="PSUM") as kvpsum:
            tr_pool[0] = ktpsum
            trp = ktpsum
            vpsum = ktpsum

            kv_sb = [kvsb_pool.tile([D + 1, 2 * M], F32R, tag=f"kv{p}", name=f"kv{p}")[:]
                     for p in range(NPAIR)]

            def emit_kvm(p):
                # kv -> m-major [m, d+1]; interleaved into the last chunk's
                # pair loop right after kv_sb[p] is finalized
                ps = trp.tile([P, 512], F32, tag="ktps", name="trp")[:]
                for j in range(4):
                    nc.tensor.transpose(
                        ps[:, j * P : j * P + (D + 1)],
                        kv_sb[p][:, j * P : (j + 1) * P].bitcast(F32),
                        ident.bitcast(F32)[0 : D + 1, 0 : D + 1],
                    )
                nc.scalar.copy(
                    kvm[p],
                    ps.rearrange("q (j c) -> q j c", c=P)[:, :, 0 : D + 1],
                )

            xt_next = [xt0]
            for ich in range(NCH):
                if 1 <= ich and ich + 1 < NCH:
                    x_prefetch(ich + 1)
                xt = xt_transposes(ich) if (PREPOLD and ich > 0) else xt_next[0]

                # qkv GEMMs in fp8 DoubleRow, 3-term compensated:
                # W stationary: out = Whi'xhi (3 hi-pair DR) + Whi'xlo +
                # Wlo'xhi (6 cross DR, slot-paired); x stationary mirrors it
                def dr_wx(out, c0, cn, last_stop):
                    if not USE_FP8:
                        for kk in range(KT):
                            nc.tensor.matmul(
                                out, _r(qkvwT[kk][:, c0 : c0 + cn]),
                                _r(xt[:, kk, :]),
                                start=(kk == 0),
                                stop=(last_stop and kk == KT - 1),
                            )
                        return
                    for j in range(KT // 2):
                        nc.tensor.matmul(
                            out,
                            wint[:, 2 * j : 2 * j + 2, 0, c0 : c0 + cn],
                            xt[:, 2 * j : 2 * j + 2, 1, :],
                            start=(j == 0), stop=False, perf_mode=DR,
                        )
                    for kk in range(KT):
                        nc.tensor.matmul(
                            out,
                            wint[:, kk, :, c0 : c0 + cn],
                            xt[:, kk, :, :],
                            start=False, stop=(last_stop and kk == KT - 1),
                            perf_mode=DR,
                        )

                def dr_xw(out, s, c0, cn, last_stop):
                    sl = slice(s * P, (s + 1) * P)
                    if not USE_FP8:
                        for kk in range(KT):
                            nc.tensor.matmul(
                                out, _r(xt[:, kk, sl]),
                                _r(qkvwT[kk][:, c0 : c0 + cn]),
                                start=(kk == 0),
                                stop=(last_stop and kk == KT - 1),
                            )
                        return
                    for j in range(KT // 2):
                        nc.tensor.matmul(
                            out,
                            xt[:, 2 * j : 2 * j + 2, 1, sl],
                            wint[:, 2 * j : 2 * j + 2, 0, c0 : c0 + cn],
                            start=(j == 0), stop=False, perf_mode=DR,
                        )
                    for kk in range(KT):
                        nc.tensor.matmul(
                            out,
                            xt[:, kk, :, sl],
                            wint[:, kk, :, c0 : c0 + cn],
                            start=False, stop=(last_stop and kk == KT - 1),
                            perf_mode=DR,
                        )

                # all kT matmuls first: ACT bias-copies trail behind PE
                kts = []

                def emit_kt(p):
                    ktps = ktpsum.tile([P, LCH], F32, tag="ktps", name="ktps")[:]
                    dr_wx(ktps, DIM + p * P, P, True)
                    kt = ktp.tile([P, LCH], F32R, tag="kt", name="kt")[:]
                    nc.scalar.activation(
                        kt, ktps, AF.Identity, bias=qkb[:, KT + p : KT + p + 1],
                        scale=SC
                    )
                    kts.append(kt)

                if ich == 0:
                    for p in range(4):
                        emit_kt(p)
                    # k blocks 4-5 (DMA'd during prep): transpose+convert now,
                    # overlapping the kT(0..3) matmuls
                    for kk in range(KT):
                        ps = tr_pool[0].tile([P, 512], F32, tag="ktps",
                                             name="trps")[:]
                        for j in range(2):
                            nc.tensor.transpose(
                                _r(ps[:, j * P : (j + 1) * P]),
                                _r(kg2[:, j, kk * P : (kk + 1) * P]),
                                _r(ident),
                            )
                        cs = slice(2 * DIM - 256, 2 * DIM)
                        nc.scalar.activation(wint[:, kk, 0, cs],
                                             ps[:, 0:256], AF.Copy, scale=WS)
                        nc.vector.scalar_tensor_tensor(
                            wint[:, kk, 1, cs], ps[:, 0:256], WS,
                            wint[:, kk, 0, cs], AL.mult, AL.subtract)
                    for p in range(4, NPAIR):
                        emit_kt(p)
                else:
                    for p in range(NPAIR):
                        emit_kt(p)

                if ich == 0 and not PREPOLD:
                    # v-section weight prep overlaps chunk-0 kT compute
                    transpose_into(qkvw_d[2 * DIM : 3 * DIM], 2 * DIM, DIM)

                # v (L-major) into the persistent ones-augmented buffer
                def emit_v(s):
                    for ci, (c0, cn) in enumerate(((0, 512), (512, 256))):
                        vps = vpsum.tile([P, 512], F32, tag="ktps", name="vps")[:]
                        dr_xw(vps[:, 0:cn], s, 2 * DIM + c0, cn, not has_qkv_b)
                        if has_qkv_b:
                            nc.tensor.matmul(
                                vps[:, 0:cn],
                                _r(ones_row),
                                _r((vb32 if USE_FP8 else vb_row)[:, c0 : c0 + cn]),
                                start=False, stop=True,
                            )
                        nc.scalar.activation(
                            vsb[:, s, 8 * ci : 8 * ci + cn // D, 0:D],
                            vps[:, 0:cn].rearrange("p (h d) -> p h d", d=D),
                            AF.Copy, scale=SC,
                        )

                if ich == 0:
                    for s in range(NSUB):
                        emit_v(s)

                if ich == 0:
                    if not PREPOLD:
                        # q-section prep before the pair loop's qT matmuls
                        transpose_into(qkvw_d[0:DIM], 0, DIM)
                    if NCH > 1:
                        x_prefetch(1)

                # next chunk's x transposes BEFORE the pair loop: the
                # boundary then starts straight into kT without copy drains
                if ich + 1 < NCH and not PREPOLD:
                    xt_next[0] = xt_transposes(ich + 1)



                # pair loop, software-pipelined: kpz(p); qT(p); kv(p-1).
                # the qT matmuls give the DVE relu+eps of kp(p) time to land
                # before kv(p) consumes it in the next iteration
                kp_tiles = [None] * NPAIR

                kpps_pend = {}

                def emit_kp_relu(p, s):
                    kpps = kpps_pend.pop((p, s))
                    kp = kp_tiles[p][s]
                    if s < K_KPACT:
                        # ACT variant: relu(z+eps) ~ relu(z)+eps (err
                        # <= eps where z<0) -- balances ACT/DVE load
                        nc.scalar.activation(kp, kpps, AF.Relu, bias=epsc)
                    else:
                        nc.vector.tensor_scalar(
                            kp, kpps, EPS, EPS, AL.add, AL.max,
                        )

                def emit_kpz(p, defer_relu=False):
                    kt = kts[p]
                    kps = []
                    for s in range(NSUB):
                        # both heads in one matmul via the block-diagonal
                        # pmT: one accumulation group, one psum bank
                        kpps = kppsum.tile([P, 2 * M], F32, tag="kpps",
                                           name="kpps")[:]
                        nc.tensor.matmul(
                            kpps,
                            _r(kt[:, s * P : (s + 1) * P]),
                            _r(pmTbd),
                            start=True, stop=True,
                        )
                        kp = kpp.tile([P, 2 * M], BF16, tag="kp", name="kp")[:]
                        kps.append(kp)
                        kpps_pend[(p, s)] = kpps
                    kp_tiles[p] = kps
                    if not defer_relu:
                        for s in range(NSUB):
                            emit_kp_relu(p, s)

                def emit_qt(p):
                    qtps = ktpsum.tile([P, LCH], F32, tag="ktps", name="qtps")[:]
                    dr_wx(qtps, p * P, P, True)
                    qtsb = qtsbp.tile([P, LCH], F32, tag="qtsb", name="qtsb")[:]
                    nc.scalar.activation(
                        qtsb, qtps, AF.Identity, bias=qkb[:, p : p + 1], scale=SC
                    )
                    nc.sync.dma_start(qtd[ich, p], qtsb)

                def emit_kv(p, alt_ps=False):
                    kps = kp_tiles[p]
                    if alt_ps:
                        # last-chunk tail: the kpz ring is drained, borrow a
                        # bank so back-to-back kvs don't serialize on the
                        # single kv bank behind the DVE adds
                        kvps = kppsum.tile([P, 2 * M], F32, tag="kpps",
                                           name="kvps")[0 : D + 1, :]
                    else:
                        kvps = kvpsum.tile([D + 1, 2 * M], F32, tag="kvps", name="kvps")[:]
                    for h2 in range(2):
                        for s in range(NSUB):
                            nc.tensor.matmul(
                                kvps[:, h2 * M : (h2 + 1) * M],
                                vsb[:, s, 2 * p + h2, :],
                                kps[s][:, h2 * M : (h2 + 1) * M],
                                start=(s == 0), stop=(s == NSUB - 1),
                            )
                    if ich == 0:
                        nc.scalar.copy(kv_sb[p], kvps)
                    else:
                        nc.vector.tensor_add(kv_sb[p], kv_sb[p], kvps)
                    kp_tiles[p] = None

                if ich == 0:
                    for p in range(NPAIR):
                        emit_kpz(p)
                        emit_qt(p)
                        if p >= K_KVLAG:
                            emit_kv(p - K_KVLAG)
                    for p in range(NPAIR - K_KVLAG, NPAIR):
                        emit_kv(p)
                    if NCH == 1:
                        for p in range(NPAIR):
                            emit_kvm(p)
                else:
                    # v subtiles interleaved with the kpz/qT pipeline: the
                    # DVE kp relus spread across the whole chunk instead of
                    # piling up in a pair-loop tail drain
                    for s in range(NSUB):
                        emit_v(s)
                        emit_kpz(s)
                        emit_qt(s)
                    emit_kv(0)
                    emit_kpz(4)
                    emit_qt(4)
                    emit_kv(1)
                    emit_kpz(5, defer_relu=(ich == NCH - 1))
                    emit_qt(5)
                    if ich == NCH - 1:
                        # last chunk: kvm transposes ride the kv tail, and
                        # pair-5's relus are deferred between the kv emissions
                        # so the kv_sb adds don't queue behind them on DVE
                        emit_kv(2)
                        emit_kvm(0)
                        emit_kp_relu(5, 0)
                        emit_kp_relu(5, 1)
                        emit_kv(3, alt_ps=K_ALTKV == 1)
                        emit_kvm(1)
                        emit_kp_relu(5, 2)
                        emit_kp_relu(5, 3)
                        emit_kv(4)
                        emit_kvm(2)
                        emit_kv(5, alt_ps=K_ALTKV == 1)
                        for p in range(3, NPAIR):
                            emit_kvm(p)
                    else:
                        for p in range(2, NPAIR):
                            emit_kv(p)

                if ich == 0:
                    if not PREPOLD:
                        # proj_w prep (pass-2 only) rides behind the rest
                        transpose_into(projw_d, 0, DIM, proj=True)
                    if QTPF_EARLY:
                        qt_prefetch(0)



            if QTPF_EARLY and NCH > 1:
                qt_prefetch(1)

    # ---- pass 2: q features, num/den, attention out, projection ----
    with tc.tile_pool(name="p2qp", bufs=10) as qpp, \
         tc.tile_pool(name="p2at", bufs=K_ATP) as atp, \
         tc.tile_pool(name="p2rd", bufs=K_RDP) as rdp, \
         tc.tile_pool(name="p2y", bufs=4) as yp, \
         tc.tile_pool(name="ps2qp", bufs=K_QPPS, space="PSUM") as qppsum, \
         tc.tile_pool(name="ps2nm", bufs=K_NMPS, space="PSUM") as numpsum, \
         tc.tile_pool(name="ps2ya", bufs=1, space="PSUM") as ypsumA, \
         tc.tile_pool(name="ps2yb", bufs=1, space="PSUM") as ypsumB:

        # division: DVE reciprocal drains den (psum row 64) to sbuf, Pool
        # broadcasts it across 64 partitions (sbuf->sbuf; Pool cannot read
        # PSUM), DVE multiply produces attn. Muls deferred so the Pool
        # broadcast lands before DVE needs it.
        pend = []

        def flush_mul(keep=0):
            while len(pend) > keep:
                p_, h2_, nmps_, rdb_, attn_ = pend.pop(0)
                nc.vector.tensor_mul(attn_[h2_ * D : (h2_ + 1) * D, p_, :],
                                     nmps_[0:D, :], rdb_)

        ysb_cur = [None]
        last_y = [False]

        def do_y_half(ich, s, ci, attn, alt=False):
            # half a y-subtile (one column segment): spread across units so
            # the PE cadence stays smooth and DVE never falls behind
            l0 = ich * LCH
            c0, cn = ((0, 512), (512, 256))[ci]
            if ci == 0:
                ysb_cur[0] = yp.tile([P, DIM], F32, tag="ysb", name="ysb")[:]
            ysb = ysb_cur[0]
            sl = slice(s * P, (s + 1) * P)
            if alt:
                # tail: borrow the idle qpz psum banks to avoid serializing
                yps = qppsum.tile([P, LCH], F32, tag="qpps",
                                  name="yps")[:, 0:cn]
            else:
                yps = (ypsumA if ci == 0 else ypsumB).tile(
                    [P, cn], F32, tag=f"yps{ci}", name="yps")[:]
            for kk in range(KT):
                nc.tensor.matmul(
                    yps,
                    _r(attn[:, kk, sl]),
                    _r(projwT[kk][:, c0 : c0 + cn]),
                    start=(kk == 0),
                    stop=(not has_proj_b and kk == KT - 1),
                )
            if has_proj_b:
                nc.tensor.matmul(
                    yps,
                    _r(ones_row),
                    _r(pb_row[:, c0 : c0 + cn]),
                    start=False, stop=True,
                )
            if ci == 0:
                if K_YC0 == 0:
                    nc.scalar.copy(ysb[:, c0 : c0 + cn], yps)
                else:
                    nc.vector.tensor_copy(ysb[:, c0 : c0 + cn], yps)
                if last_y[0]:
                    nc.sync.dma_start(
                        y_d[l0 + s * P : l0 + (s + 1) * P, 0:512],
                        ysb[:, 0:512])
            else:
                if K_YC1 == 0:
                    nc.vector.tensor_copy(ysb[:, c0 : c0 + cn], yps)
                else:
                    nc.scalar.copy(ysb[:, c0 : c0 + cn], yps)
                if last_y[0]:
                    nc.sync.dma_start(
                        y_d[l0 + s * P : l0 + (s + 1) * P, 512:DIM],
                        ysb[:, 512:DIM])
                else:
                    nc.sync.dma_start(
                        y_d[l0 + s * P : l0 + (s + 1) * P, :], ysb)

        def do_y_subtile(ich, s, attn):
            do_y_half(ich, s, 0, attn)
            do_y_half(ich, s, 1, attn)


        def do_pairs_chunked(ich, prev_attn):
            attn = atp.tile([P, NPAIR, LCH], F32R, tag="attn", name="attn")[:]
            qt_cur = {p: qt_tiles.pop((ich, p)) for p in range(NPAIR)}
            rq = []
            units = [(p, h2) for p in range(NPAIR) for h2 in range(2)]
            qpsl = {}

            def emit_qpz_c(u):
                p, h2 = u
                r0 = h2 * D
                qp = []
                for mt in range(2):
                    qpsum = qppsum.tile([P, LCH], F32, tag="qpps", name="qpps")[:]
                    nc.tensor.matmul(
                        qpsum,
                        _r(pmT[r0 : r0 + D, mt * P : (mt + 1) * P]),
                        _r(qt_cur[p][r0 : r0 + D, :]),
                        start=True, stop=True,
                    )
                    t = qpp.tile([P, LCH], F32R, tag="qp", name="qp")[:]
                    nc.scalar.activation(t, qpsum, AF.Relu, bias=epsc)
                    qp.append(t)
                qpsl[u] = qp

            emit_qpz_c(units[0])
            for i, u in enumerate(units):
                if i + 1 < len(units):
                    emit_qpz_c(units[i + 1])
                p, h2 = u
                qp = qpsl.pop(u)
                nmps = numpsum.tile([D + 1, LCH], F32, tag="nmps", name="nmps")[:]
                for mt in range(2):
                    nc.tensor.matmul(
                        nmps,
                        _r(kvm[p][:, 2 * h2 + mt, :]),
                        _r(qp[mt]),
                        start=(mt == 0), stop=(mt == 1),
                    )
                rq.append((p, h2, nmps))
                eff_def = 1 if ich == NCH - 1 else K_DEF
                if len(rq) > eff_def:
                    p_, h2_, nm_prev = rq.pop(0)
                    flush_mul(keep=0 if ich == NCH - 1 else K_KEEP)
                    rd = rdp.tile([1, LCH], F32, tag="rd", name="rd")[:]
                    nc.vector.reciprocal(rd, nm_prev[D : D + 1, :])
                    rdb = rdp.tile([D, LCH], F32, tag="rdb", name="rdb")[:]
                    nc.gpsimd.partition_broadcast(rdb, rd, channels=D)
                    pend.append((p_, h2_, nm_prev, rdb, attn))
                if prev_attn is not None:
                    yi = 2 * p + h2 - K_YOFF
                    if 0 <= yi < 2 * NSUB:
                        do_y_half(ich - 1, yi // 2, yi % 2, prev_attn)
            while rq:
                p_, h2_, nm_prev = rq.pop(0)
                rd = rdp.tile([1, LCH], F32, tag="rd", name="rd")[:]
                nc.vector.reciprocal(rd, nm_prev[D : D + 1, :])
                rdb = rdp.tile([D, LCH], F32, tag="rdb", name="rdb")[:]
                nc.gpsimd.partition_broadcast(rdb, rd, channels=D)
                pend.append((p_, h2_, nm_prev, rdb, attn))
            flush_mul()
            return attn

        if int(os.environ.get("K_FLAT", "0")) == 0:
            if not QTPF_EARLY:
                qt_prefetch(0)
                if NCH > 1:
                    qt_prefetch(1)
            prev = None
            for ich in range(NCH):
                if ich + 2 < NCH:
                    qt_prefetch(ich + 2)
                prev = do_pairs_chunked(ich, prev)
            for s in range(NSUB):
                last_y[0] = (s == NSUB - 1)
                do_y_half(NCH - 1, s, 0, prev, alt=(s % 2 == 1))
                do_y_half(NCH - 1, s, 1, prev, alt=(s % 2 == 0))
            return
        # pass 2 flattened into one unit stream across all chunks: the qpz
        # lookahead and the divide deferral roll over chunk boundaries, so
        # no per-chunk pipeline fill/drain
        attn_of = {}
        qt_cur_all = {}
        qps = {}
        rqueue = []

        def ensure_chunk(ich):
            if ich in attn_of:
                return
            attn_of[ich] = atp.tile([P, NPAIR, LCH], F32R, tag="attn",
                                    name="attn")[:]
            qt_cur_all[ich] = {p: qt_tiles.pop((ich, p)) for p in range(NPAIR)}

        def emit_qpz(iu):
            ich, p, h2 = iu
            ensure_chunk(ich)
            r0 = h2 * D
            qp = []
            for mt in range(2):
                qpsum = qppsum.tile([P, LCH], F32, tag="qpps", name="qpps")[:]
                nc.tensor.matmul(
                    qpsum,
                    _r(pmT[r0 : r0 + D, mt * P : (mt + 1) * P]),
                    _r(qt_cur_all[ich][p][r0 : r0 + D, :]),
                    start=True, stop=True,
                )
                t = qpp.tile([P, LCH], F32R, tag="qp", name="qp")[:]
                # q_p = relu(z+eps) ~ reference's relu(z)+eps (the bias
                # rides free on the ACT op; residual error <= eps)
                nc.scalar.activation(t, qpsum, AF.Relu, bias=epsc)
                qp.append(t)
            qps[iu] = qp

        def drain_rqueue(eff_def, keep):
            while len(rqueue) > eff_def:
                ic_, p_, h2_, nm_prev = rqueue.pop(0)
                flush_mul(keep=keep)
                rd = rdp.tile([1, LCH], F32, tag="rd", name="rd")[:]
                nc.vector.reciprocal(rd, nm_prev[D : D + 1, :])
                rdb = rdp.tile([D, LCH], F32, tag="rdb", name="rdb")[:]
                nc.gpsimd.partition_broadcast(rdb, rd, channels=D)
                pend.append((p_, h2_, nm_prev, rdb, attn_of[ic_]))

        if not QTPF_EARLY:
            qt_prefetch(0)
            if NCH > 1:
                qt_prefetch(1)
        units_all = [(ich, p, h2) for ich in range(NCH)
                     for p in range(NPAIR) for h2 in range(2)]
        emit_qpz(units_all[0])
        for i, iu in enumerate(units_all):
            ich, p, h2 = iu
            if p == 0 and h2 == 0 and ich + 2 < NCH:
                qt_prefetch(ich + 2)
            if i + 1 < len(units_all):
                emit_qpz(units_all[i + 1])
            r0 = h2 * D
            qp = qps.pop(iu)
            nmps = numpsum.tile([D + 1, LCH], F32, tag="nmps", name="nmps")[:]
            for mt in range(2):
                nc.tensor.matmul(
                    nmps,
                    _r(kvm[p][:, 2 * h2 + mt, :]),
                    _r(qp[mt]),
                    start=(mt == 0), stop=(mt == 1),
                )
            rqueue.append((ich, p, h2, nmps))
            last_ch = ich == NCH - 1
            drain_rqueue(1 if last_ch else K_DEF, 0 if last_ch else K_KEEP)
            # the first y-half of a chunk (unit K_YOFF) reads ALL of the
            # previous chunk's attn: ramp the division pipeline down over
            # the first units so its muls are all emitted by then
            uic = 2 * p + h2
            if ich > 0 and uic < K_YOFF:
                drain_rqueue(K_YOFF - 1 - uic, 0)
            # y-projection of the previous chunk as PE filler, half a
            # subtile per unit
            if ich > 0:
                yi = 2 * p + h2 - K_YOFF
                if 0 <= yi < 2 * NSUB:
                    do_y_half(ich - 1, yi // 2, yi % 2, attn_of[ich - 1])
        drain_rqueue(0, 0)
        flush_mul()
        prev = attn_of[NCH - 1]
        for s in range(NSUB):
            last_y[0] = (s == NSUB - 1)
            do_y_half(NCH - 1, s, 0, prev, alt=(s % 2 == 1))
            do_y_half(NCH - 1, s, 1, prev, alt=(s % 2 == 0))


_CACHE = {}


def _get_nc(L=4096, hqb=True, hpb=True):
    key = ("nc", L, hqb, hpb)
    if key not in _CACHE:
        _CACHE[key] = build(L, hqb, hpb)
    return _CACHE[key]


last_exec_time_ns = None
last_profile = None


def kernel(x, qkv_w, qkv_b, proj_w, proj_b, proj_mat):
    global last_exec_time_ns, last_profile
    from concourse.bass_utils import run_bass_kernel_spmd

    x = np.asarray(x, np.float32)
    B, L, _ = x.shape
    hqb = bool(np.any(np.asarray(qkv_b)))
    hpb = bool(np.any(np.asarray(proj_b)))
    nc = _get_nc(L, hqb, hpb)
    base = {
        "qkv_w": np.ascontiguousarray(np.asarray(qkv_w, np.float32)),
        "qkv_b": np.ascontiguousarray(np.asarray(qkv_b, np.float32)),
        "proj_w": np.ascontiguousarray(np.asarray(proj_w, np.float32)),
        "proj_b": np.ascontiguousarray(np.asarray(proj_b, np.float32)),
        "proj_mat": np.ascontiguousarray(np.asarray(proj_mat, np.float32)),
    }
    in_maps = [dict(base, x=np.ascontiguousarray(x[b])) for b in range(B)]
    trace = bool(int(os.environ.get("KERNEL_TRACE", "0")))
    res = run_bass_kernel_spmd(nc, in_maps, core_ids=list(range(B)), trace=trace)
    last_exec_time_ns = res.exec_time_ns
    last_profile = res.profile_json
    return np.stack([res.results[b]["y"] for b in range(B)], axis=0)


if __name__ == "__main__":
    # CoreSim smoke test at reduced L
    from concourse.bass_interp import CoreSim

    Ls = int(os.environ.get("SIM_L", "512"))
    rng = np.random.default_rng(0)
    x = rng.standard_normal((Ls, DIM), dtype=np.float32)
    qkv_w = (rng.standard_normal((3 * DIM, DIM), dtype=np.float32) * DIM**-0.5)
    qkv_b = rng.standard_normal(3 * DIM, dtype=np.float32) * 0.1
    proj_w = (rng.standard_normal((DIM, DIM), dtype=np.float32) * DIM**-0.5)
    proj_b = rng.standard_normal(DIM, dtype=np.float32) * 0.1
    proj_mat = rng.standard_normal((M, D), dtype=np.float32)

    def ref_np(x, qkv_w, qkv_b, proj_w, proj_b, proj_mat):
        qkv = x @ qkv_w.T + qkv_b
        qkv = qkv.reshape(Ls, 3, H, D)
        q, k, v = qkv[:, 0], qkv[:, 1], qkv[:, 2]
        qp = np.maximum(RATIO * np.einsum("lhd,md->lhm", q, proj_mat), 0) + EPS
        kp = np.maximum(RATIO * np.einsum("lhd,md->lhm", k, proj_mat), 0) + EPS
        kv = np.einsum("lhm,lhd->hmd", kp, v)
        ks = kp.sum(axis=0)
        num = np.einsum("lhm,hmd->lhd", qp, kv)
        den = np.einsum("lhm,hm->lh", qp, ks)
        out = (num / den[..., None]).reshape(Ls, DIM)
        return out @ proj_w.T + proj_b

    print(f"building L={Ls} ...")
    nc = build(Ls)
    print("simulating ...")
    sim = CoreSim(nc)
    for name, arr in [("x", x), ("qkv_w", qkv_w), ("qkv_b", qkv_b),
                      ("proj_w", proj_w), ("proj_b", proj_b),
                      ("proj_mat", proj_mat)]:
        sim.tensor(name)[:] = arr
    sim.simulate(check_with_hw=False)
    got = np.array(sim.tensor("y"))
    want = ref_np(x, qkv_w, qkv_b, proj_w, proj_b, proj_mat)
    err = np.abs(got - want)
    rel = np.linalg.norm(got - want) / np.linalg.norm(want)
    print("max abs err:", err.max(), " rel fro err:", rel)
    assert rel < 2e-2, "sim mismatch"
    print("SIM OK")

